# revision 17
# baseline (speedup 1.0000x reference)
"""Trainium2 Bass kernel for nn_CrossAttention (dense_transformer).

Reference computation (per batch b, per stream s in {1,2}):
    q_s   = heads(x_s)                      # [H, N, D] slices of x_s
    kv_s  = x_s @ Wkv_s -> k_s, v_s         # [N, C] each
    gate_s= sigmoid(relu(x_s @ w1 + b1) @ w2 + b2)
    ctx_s = softmax_d( scale * k_s^T @ (v_s * gate_s) )   # [H, D, D], softmax over d
    o_1   = q_1 @ ctx_2 ; o_2 = q_2 @ ctx_1  (cross)

Sharding: 8 cores = (stream s, batch b) pairs.  Core (s, b) projects
x_s[b] (kv + gate + ctx_s[b]) and then computes the OTHER stream's
output o_{1-s}[b] = q_{1-s}[b] @ softmax(ctx_s[b]).  No cross-core
communication; host concatenates outputs.

v2: host pre-transposes and fp16-casts x (so no on-chip transposes),
fp16 matmul operands everywhere (fp32 PSUM accumulate), fused
block-pipelined gate1 -> gate2/kv/vg -> ctx (PSUM-accumulated across
all 32 n-chunks), softmax, then phase B streaming xqT.
"""

import numpy as np
from contextlib import ExitStack

N = 4096
C = 1024
H = 16
D = 64
SCALE = D ** (-0.5)
NBLK = 4            # n-blocks of 1024 rows
BN = N // NBLK      # 1024 rows per block
BCH = BN // 128     # 8 chunks of 128 rows per block

_CACHE = {}


def _build_program(with_bias):
    """Build the SPMD Bass program (same for all 8 cores)."""
    import concourse.bass as bass
    import concourse.bacc as bacc
    import concourse.tile as tile
    import concourse.mybir as mybir

    F32 = mybir.dt.float32
    F16 = mybir.dt.float16
    AF = mybir.ActivationFunctionType

    nc = bacc.Bacc("TRN2", target_bir_lowering=False, debug=False, num_devices=8)

    xpT = nc.dram_tensor("xpT", [C, N], F16, kind="ExternalInput").ap()
    xqT = nc.dram_tensor("xqT", [C, N], F16, kind="ExternalInput").ap()
    wkv = nc.dram_tensor("wkv", [C, 2 * C], F16, kind="ExternalInput").ap()
    w1 = nc.dram_tensor("w1", [C, C], F16, kind="ExternalInput").ap()
    w2 = nc.dram_tensor("w2", [C, C], F16, kind="ExternalInput").ap()
    b1 = nc.dram_tensor("b1", [C], F32, kind="ExternalInput").ap()
    b2 = nc.dram_tensor("b2", [C], F16, kind="ExternalInput").ap()
    identh = nc.dram_tensor("identh", [128, 128], F16, kind="ExternalInput").ap()
    maskh = nc.dram_tensor("maskh", [128, 128], F16, kind="ExternalInput").ap()
    # output is produced TRANSPOSED ([channel, n]); host un-transposes.
    o = nc.dram_tensor("o", [C, N], F16, kind="ExternalOutput").ap()

    with tile.TileContext(nc) as tc, ExitStack() as ctx:
        # ---------- persistent pools ----------
        cpool = ctx.enter_context(tc.tile_pool(name="consts", bufs=1))
        b1_sb = cpool.tile([128, 8], F32, name="b1_sb")  # b1_sb[p, m] = b1[m*128+p]
        nc.sync.dma_start(b1_sb, b1.rearrange("(m p) -> p m", p=128))
        identh_sb = cpool.tile([128, 128], F16, name="identh_sb")
        nc.sync.dma_start(identh_sb, identh)
        maskh_sb = cpool.tile([128, 128], F16, name="maskh_sb")
        nc.sync.dma_start(maskh_sb, maskh)
        if with_bias:
            ones_r = cpool.tile([1, 128], F16, name="ones_r")
            nc.vector.memset(ones_r, 1.0)
            b2_r = cpool.tile([1, C], F16, name="b2_r")
            nc.sync.dma_start(b2_r, b2.rearrange("(one f) -> one f", one=1))

        spool = ctx.enter_context(tc.tile_pool(name="spairs", bufs=1))
        spairs = [spool.tile([128, 128], F16, name=f"spair{j}") for j in range(8)]

        # weights: w1 first (gate1 is the critical path at startup), split in
        # m-halves so gate1 m=0 can start before the whole matrix lands.
        wpool = ctx.enter_context(tc.tile_pool(name="weights", bufs=1))
        w1r = w1.rearrange("(k p) m -> p k m", p=128)
        w1_sb = [wpool.tile([128, 8, C // 2], F16, name=f"w1_sb{h}")
                 for h in range(2)]
        nc.sync.dma_start(w1_sb[0], w1r[:, :, 0:C // 2])

        # ctx^T accumulator in SBUF, [e(2-head pair), d(2-head pair)] per pair j
        # at cols j*128; per-chunk partials land in a PSUM tile and are
        # DVE-accumulated here (one-shot matmul groups, like the baseline).
        acc_pool = ctx.enter_context(tc.tile_pool(name="ctxacc", bufs=1))
        ctx_acc = acc_pool.tile([128, 1024], F32, name="ctx_acc")
        nc.vector.memset(ctx_acc, 0.0)

        # xqT prefetch pool (phase B input), fully resident by end of phase A
        # so phase B's DMA is writes-only (reads+writes together oversubscribe
        # the 358 GB/s DMA during the short phase B window).
        xqt_pool = ctx.enter_context(tc.tile_pool(name="xqt", bufs=4))
        xqt_tiles = {}

        def load_xqt(blk):
            t = xqt_pool.tile([128, 8, BN], F16, name="xqt", tag="xqt")
            nc.sync.dma_start(
                t,
                xqT.rearrange("(k p) n -> p k n", p=128)[
                    :, :, blk * BN:(blk + 1) * BN
                ],
            )
            xqt_tiles[blk] = t

        # =========================================================
        # Phase A: gate MLP + kv projection + ctx accumulation,
        # fused per n-block of 1024 rows.
        # =========================================================
        with ExitStack() as pa:
            # xpT in half-block tiles of 512 n-cols each so the first gate1
            # psum group starts as soon as the first half lands.
            xpt_pool = pa.enter_context(tc.tile_pool(name="xpt", bufs=4))
            xpt_tiles = {}
            xpTr = xpT.rearrange("(k p) n -> p k n", p=128)

            def load_xpt(blk, h):
                t = xpt_pool.tile([128, 8, BN // 2], F16, name="xpt", tag="xpt")
                lo = blk * BN + h * (BN // 2)
                nc.sync.dma_start(t, xpTr[:, :, lo:lo + BN // 2])
                xpt_tiles[(blk, h)] = t

            load_xpt(0, 0)
            nc.sync.dma_start(w1_sb[1], w1r[:, :, C // 2:])
            load_xpt(0, 1)
            # remaining big DMAs, in priority order behind xpt block 0
            wkv_sb = wpool.tile([128, 8, 2 * C], F16, name="wkv_sb")
            nc.sync.dma_start(wkv_sb, wkv.rearrange("(k p) m -> p k m", p=128))
            w2_sb = wpool.tile([128, 8, C], F16, name="w2_sb")
            nc.sync.dma_start(w2_sb, w2.rearrange("(k p) m -> p k m", p=128))
            load_xqt(0)
            load_xqt(1)

            ht_pool = pa.enter_context(tc.tile_pool(name="ht", bufs=1))
            g_pool = pa.enter_context(tc.tile_pool(name="g", bufs=3))
            k_pool = pa.enter_context(tc.tile_pool(name="k", bufs=3))
            vg_pool = pa.enter_context(tc.tile_pool(name="vg", bufs=3))
            g1ps_pool = pa.enter_context(
                tc.tile_pool(name="g1ps", bufs=2, space="PSUM")
            )
            g2ps_pool = pa.enter_context(
                tc.tile_pool(name="g2ps", bufs=2, space="PSUM")
            )
            kvps_pool = pa.enter_context(
                tc.tile_pool(name="kvps", bufs=2, space="PSUM")
            )
            ctps_pool = pa.enter_context(
                tc.tile_pool(name="ctps", bufs=1, space="PSUM")
            )

            # ctx matmuls are emitted one chunk late so their vector-produced
            # inputs (k, vg) are ready by the time PE reaches them.
            pending = []

            def emit_ctx():
                if not pending:
                    return
                k_sb, vg, nch = pending.pop(0)
                ctp = ctps_pool.tile([128, 1024], F32, name="ctp", tag="ctp")
                for j in range(8):
                    nc.tensor.matmul(
                        ctp[:, j * 128:(j + 1) * 128],
                        vg[:, j * 128:(j + 1) * 128],
                        k_sb[:, j * 128:(j + 1) * 128],
                        start=True,
                        stop=True,
                        skip_group_check=True,
                    )
                nc.vector.tensor_add(ctx_acc, ctx_acc, ctp)

            for blk in range(NBLK):
                if blk + 1 < NBLK:
                    load_xpt(blk + 1, 0)
                    load_xpt(blk + 1, 1)
                if blk + 2 < NBLK:
                    load_xqt(blk + 2)
                xpth = [xpt_tiles.pop((blk, h)) for h in range(2)]
                # gate1: hT[m, n] = relu((xp @ w1 + b1).T), w1 stationary
                ht = ht_pool.tile([128, 8, BN], F16, name="ht", tag="ht")
                for m in range(8):
                    for half in range(2):
                        ps = g1ps_pool.tile([128, 512], F32, name="g1ps", tag="g1ps")
                        for kk in range(8):
                            nc.tensor.matmul(
                                ps,
                                w1_sb[m // 4][:, kk, (m % 4) * 128:(m % 4 + 1) * 128],
                                xpth[half][:, kk, :],
                                start=(kk == 0),
                                stop=(kk == 7),
                            )
                        nc.scalar.activation(
                            ht[:, m, half * 512:(half + 1) * 512],
                            ps,
                            AF.Relu,
                            bias=b1_sb[:, m:m + 1],
                        )
                for ch in range(BCH):
                    nch = blk * BCH + ch
                    xpc_t = xpth[ch // 4]
                    xc0 = (ch % 4) * 128
                    # gate2: g[n, q] = sigmoid(h @ w2 + b2), hT stationary
                    g = g_pool.tile([128, C], F16, name="g", tag="g")
                    for half in range(2):
                        ps = g2ps_pool.tile([128, 512], F32, name="g2ps", tag="g2ps")
                        for kk in range(8):
                            nc.tensor.matmul(
                                ps,
                                ht[:, kk, ch * 128:(ch + 1) * 128],
                                w2_sb[:, kk, half * 512:(half + 1) * 512],
                                start=(kk == 0),
                                stop=(kk == 7 and not with_bias),
                            )
                        if with_bias:
                            nc.tensor.matmul(
                                ps,
                                ones_r,
                                b2_r[:, half * 512:(half + 1) * 512],
                                start=False,
                                stop=True,
                            )
                        nc.scalar.activation(
                            g[:, half * 512:(half + 1) * 512], ps, AF.Sigmoid
                        )
                    # kv projection: kv[n, m], xpT stationary
                    k_sb = k_pool.tile([128, C], F16, name="k_sb", tag="k_sb")
                    vg = vg_pool.tile([128, C], F16, name="vg", tag="vg")
                    for q in range(4):
                        ps = kvps_pool.tile([128, 512], F32, name="kvps", tag="kvps")
                        for kk in range(8):
                            nc.tensor.matmul(
                                ps,
                                xpc_t[:, kk, xc0:xc0 + 128],
                                wkv_sb[:, kk, q * 512:(q + 1) * 512],
                                start=(kk == 0),
                                stop=(kk == 7),
                            )
                        if q < 2:
                            nc.vector.tensor_copy(k_sb[:, q * 512:(q + 1) * 512], ps)
                        else:
                            qq = q - 2
                            nc.vector.tensor_mul(
                                vg[:, qq * 512:(qq + 1) * 512],
                                ps,
                                g[:, qq * 512:(qq + 1) * 512],
                            )
                    emit_ctx()
                    pending.append((k_sb, vg, nch))
            emit_ctx()

        # =========================================================
        # Softmax over d (free dim of ctx^T) + build block-diag S pairs
        # =========================================================
        with ExitStack() as sm:
            smp = sm.enter_context(tc.tile_pool(name="smpool", bufs=1))
            smps = sm.enter_context(tc.tile_pool(name="smps", bufs=2, space="PSUM"))
            maxs = smp.tile([128, 16], F32, name="maxs")
            nc.vector.tensor_reduce(
                maxs,
                ctx_acc.rearrange("p (g d) -> p g d", g=16),
                axis=mybir.AxisListType.X,
                op=mybir.AluOpType.max,
            )
            # exp(scale*(x - max)) fused on ScalarE: bias = -scale*max per
            # 64-col group, accum_out gives the per-group sums for free.
            nmaxs = smp.tile([128, 16], F32, name="nmaxs")
            nc.scalar.mul(nmaxs, maxs, -float(SCALE))
            et = smp.tile([128, 1024], F32, name="et")
            sums = smp.tile([128, 16], F32, name="sums")
            for gi in range(16):
                nc.scalar.activation(
                    et[:, gi * 64:(gi + 1) * 64],
                    ctx_acc[:, gi * 64:(gi + 1) * 64],
                    AF.Exp,
                    scale=float(SCALE),
                    bias=nmaxs[:, gi:gi + 1],
                    accum_out=sums[:, gi:gi + 1],
                )
            recs = smp.tile([128, 16], F32, name="recs")
            nc.vector.reciprocal(recs, sums)
            stb = smp.tile([128, 1024], F16, name="stb")
            nc.vector.tensor_mul(
                stb.rearrange("p (g d) -> p g d", g=16),
                et.rearrange("p (g d) -> p g d", g=16),
                recs.unsqueeze(-1).broadcast_to([128, 16, 64]),
            )
            # stb[:, j*128:(j+1)*128] = softmaxed ctxT pair [e(2), d(2)];
            # transpose -> [d(2), e(2)], mask off the off-diagonal garbage.
            for j in range(8):
                tp = smps.tile([128, 128], F16, name="smtp", tag="smtp")
                nc.tensor.transpose(
                    tp, stb[:, j * 128:(j + 1) * 128], identh_sb
                )
                nc.vector.tensor_mul(spairs[j], tp, maskh_sb)

        # =========================================================
        # Phase B (transposed out): oT[j*128:(j+1)*128, nblk] =
        #   spair_j^T @ xqT[j-pair rows, nblk].  spair_j stays stationary
        #   across all n (8 LDWEIGHTS total), xqT streams 1024 cols/MM.
        # =========================================================
        with ExitStack() as pb:
            oo_pool = pb.enter_context(tc.tile_pool(name="oo", bufs=4))
            bops_pool = pb.enter_context(
                tc.tile_pool(name="bops", bufs=4, space="PSUM")
            )
            for j in range(8):
                for blk in range(NBLK):
                    xqt = xqt_tiles[blk]
                    ops = bops_pool.tile([128, BN], F32, name="ops", tag="ops")
                    nc.tensor.matmul(
                        ops,
                        spairs[j],
                        xqt[:, j, :],
                        start=True,
                        stop=True,
                    )
                    oo = oo_pool.tile([128, BN], F16, name="oo", tag="oo")
                    if blk % 2 == 0:
                        nc.vector.tensor_copy(oo, ops)
                    else:
                        nc.scalar.copy(oo, ops)
                    nc.sync.dma_start(
                        o[j * 128:(j + 1) * 128, blk * BN:(blk + 1) * BN], oo
                    )

    nc.compile()
    return nc


def _get_program(with_bias=False):
    key = ("nc", bool(with_bias))
    if key not in _CACHE:
        _CACHE[key] = _build_program(with_bias)
    return _CACHE[key]


def make_in_maps(x1, x2, Wkv1, Wkv2, g1_w1, g1_b1, g1_w2, g1_b2,
                 g2_w1, g2_b1, g2_w2, g2_b2):
    """Core (s, b): cores 0-3 = (s=0, b), cores 4-7 = (s=1, b)."""
    f16 = np.float16
    ident = np.eye(128, dtype=f16)
    mask = np.zeros((128, 128), dtype=f16)
    mask[:64, :64] = np.float16(1.0)
    mask[64:, 64:] = np.float16(1.0)
    # transposed fp16 copies of each batch of each stream (shared across cores)
    x1T = [np.asarray(x1[b], np.float32).T.astype(f16) for b in range(x1.shape[0])]
    x2T = [np.asarray(x2[b], np.float32).T.astype(f16) for b in range(x2.shape[0])]
    wkv1h = np.asarray(Wkv1, np.float32).astype(f16)
    wkv2h = np.asarray(Wkv2, np.float32).astype(f16)
    w11h = np.asarray(g1_w1, np.float32).astype(f16)
    w12h = np.asarray(g1_w2, np.float32).astype(f16)
    w21h = np.asarray(g2_w1, np.float32).astype(f16)
    w22h = np.asarray(g2_w2, np.float32).astype(f16)
    b11 = np.asarray(g1_b1, np.float32)
    b21 = np.asarray(g2_b1, np.float32)
    b12h = np.asarray(g1_b2, np.float32).astype(f16)
    b22h = np.asarray(g2_b2, np.float32).astype(f16)
    in_maps = []
    for core in range(8):
        s, b = core // 4, core % 4
        if s == 0:
            m = dict(xpT=x1T[b], xqT=x2T[b], wkv=wkv1h,
                     w1=w11h, b1=b11, w2=w12h, b2=b12h)
        else:
            m = dict(xpT=x2T[b], xqT=x1T[b], wkv=wkv2h,
                     w1=w21h, b1=b21, w2=w22h, b2=b22h)
        m["identh"] = ident
        m["maskh"] = mask
        in_maps.append(m)
    return in_maps


def kernel(x1, x2, Wkv1, Wkv2, g1_w1, g1_b1, g1_w2, g1_b2,
           g2_w1, g2_b1, g2_w2, g2_b2, _runner=None):
    """Full-input entry point.  Returns (o1, o2), each [4, 4096, 1024] f32."""
    from concourse.bass_utils import run_bass_kernel_spmd

    args = [np.asarray(a, dtype=np.float32) for a in
            (x1, x2, Wkv1, Wkv2, g1_w1, g1_b1, g1_w2, g1_b2,
             g2_w1, g2_b1, g2_w2, g2_b2)]
    with_bias = bool(np.any(args[7]) or np.any(args[11]))  # g1_b2, g2_b2
    nc = _get_program(with_bias)
    in_maps = make_in_maps(*args)
    if _runner is None:
        res = run_bass_kernel_spmd(nc, in_maps, core_ids=list(range(8)))
        results = res.results
    else:
        results = _runner(nc, in_maps)

    B = x1.shape[0]
    o1 = np.empty((B, N, C), dtype=np.float32)
    o2 = np.empty((B, N, C), dtype=np.float32)
    for core in range(8):
        s, b = core // 4, core % 4
        out = np.asarray(results[core]["o"], dtype=np.float32)
        if s == 0:
            o2[b] = out   # core projected x1 -> ctx1 -> o2 = q2 @ ctx1
        else:
            o1[b] = out
    return (o1, o2)


# revision 20
# speedup vs baseline: 1.2120x; 1.2120x over previous
"""Trainium2 Bass kernel for nn_CrossAttention (dense_transformer).

Reference computation (per batch b, per stream s in {1,2}):
    q_s   = heads(x_s)                      # [H, N, D] slices of x_s
    kv_s  = x_s @ Wkv_s -> k_s, v_s         # [N, C] each
    gate_s= sigmoid(relu(x_s @ w1 + b1) @ w2 + b2)
    ctx_s = softmax_d( scale * k_s^T @ (v_s * gate_s) )   # [H, D, D], softmax over d
    o_1   = q_1 @ ctx_2 ; o_2 = q_2 @ ctx_1  (cross)

Sharding: 8 cores = (stream s, batch b) pairs.  Core (s, b) projects
x_s[b] (kv + gate + ctx_s[b]) and then computes the OTHER stream's
output o_{1-s}[b] = q_{1-s}[b] @ softmax(ctx_s[b]).  No cross-core
communication; host concatenates outputs.

v2: host pre-transposes and fp16-casts x (so no on-chip transposes),
fp16 matmul operands everywhere (fp32 PSUM accumulate), fused
block-pipelined gate1 -> gate2/kv/vg -> ctx (PSUM-accumulated across
all 32 n-chunks), softmax, then phase B streaming xqT.
"""

import numpy as np
from contextlib import ExitStack

N = 4096
C = 1024
H = 16
D = 64
SCALE = D ** (-0.5)
NBLK = 4            # n-blocks of 1024 rows
BN = N // NBLK      # 1024 rows per block
BCH = BN // 128     # 8 chunks of 128 rows per block

_CACHE = {}


def _build_program(with_bias):
    """Build the SPMD Bass program (same for all 8 cores)."""
    import concourse.bass as bass
    import concourse.bacc as bacc
    import concourse.tile as tile
    import concourse.mybir as mybir

    F32 = mybir.dt.float32
    F16 = mybir.dt.float16
    AF = mybir.ActivationFunctionType

    nc = bacc.Bacc("TRN2", target_bir_lowering=False, debug=False, num_devices=8)

    xpT = nc.dram_tensor("xpT", [C, N], F16, kind="ExternalInput").ap()
    xqT = nc.dram_tensor("xqT", [C, N], F16, kind="ExternalInput").ap()
    wkv = nc.dram_tensor("wkv", [C, 2 * C], F16, kind="ExternalInput").ap()
    w1 = nc.dram_tensor("w1", [C, C], F16, kind="ExternalInput").ap()
    w2 = nc.dram_tensor("w2", [C, C], F16, kind="ExternalInput").ap()
    b1 = nc.dram_tensor("b1", [C], F32, kind="ExternalInput").ap()
    b2 = nc.dram_tensor("b2", [C], F16, kind="ExternalInput").ap()
    identh = nc.dram_tensor("identh", [128, 128], F16, kind="ExternalInput").ap()
    maskh = nc.dram_tensor("maskh", [128, 128], F16, kind="ExternalInput").ap()
    # output is produced TRANSPOSED ([channel, n]); host un-transposes.
    o = nc.dram_tensor("o", [C, N], F16, kind="ExternalOutput").ap()

    with tile.TileContext(nc) as tc, ExitStack() as ctx:
        # ---------- persistent pools ----------
        cpool = ctx.enter_context(tc.tile_pool(name="consts", bufs=1))
        b1_sb = cpool.tile([128, 8], F32, name="b1_sb")  # b1_sb[p, m] = b1[m*128+p]
        nc.sync.dma_start(b1_sb, b1.rearrange("(m p) -> p m", p=128))
        identh_sb = cpool.tile([128, 128], F16, name="identh_sb")
        nc.sync.dma_start(identh_sb, identh)
        maskh_sb = cpool.tile([128, 128], F16, name="maskh_sb")
        nc.sync.dma_start(maskh_sb, maskh)
        if with_bias:
            ones_r = cpool.tile([1, 128], F16, name="ones_r")
            nc.vector.memset(ones_r, 1.0)
            b2_r = cpool.tile([1, C], F16, name="b2_r")
            nc.sync.dma_start(b2_r, b2.rearrange("(one f) -> one f", one=1))

        spool = ctx.enter_context(tc.tile_pool(name="spairs", bufs=1))
        spairs = [spool.tile([128, 128], F16, name=f"spair{j}") for j in range(8)]

        # weights: w1 first (gate1 is the critical path at startup), split in
        # m-halves so gate1 m=0 can start before the whole matrix lands.
        wpool = ctx.enter_context(tc.tile_pool(name="weights", bufs=1))
        w1r = w1.rearrange("(k p) m -> p k m", p=128)
        w1_sb = [wpool.tile([128, 8, C // 2], F16, name=f"w1_sb{h}")
                 for h in range(2)]
        nc.sync.dma_start(w1_sb[0], w1r[:, :, 0:C // 2])

        # ctx^T accumulator in SBUF, [e(2-head pair), d(2-head pair)] per pair j
        # at cols j*128; per-chunk partials land in a PSUM tile and are
        # DVE-accumulated here (one-shot matmul groups, like the baseline).
        acc_pool = ctx.enter_context(tc.tile_pool(name="ctxacc", bufs=1))
        ctx_acc = acc_pool.tile([128, 1024], F32, name="ctx_acc")
        nc.vector.memset(ctx_acc, 0.0)

        # xqT prefetch pool (phase B input), fully resident by end of phase A
        # so phase B's DMA is writes-only (reads+writes together oversubscribe
        # the 358 GB/s DMA during the short phase B window).
        xqt_pool = ctx.enter_context(tc.tile_pool(name="xqt", bufs=4))
        xqt_tiles = {}

        def load_xqt(blk):
            t = xqt_pool.tile([128, 8, BN], F16, name="xqt", tag="xqt")
            nc.sync.dma_start(
                t,
                xqT.rearrange("(k p) n -> p k n", p=128)[
                    :, :, blk * BN:(blk + 1) * BN
                ],
            )
            xqt_tiles[blk] = t

        # =========================================================
        # Phase A: gate MLP + kv projection + ctx accumulation,
        # fused per n-block of 1024 rows.
        # =========================================================
        with ExitStack() as pa:
            # xpT in half-block tiles of 512 n-cols each so the first gate1
            # psum group starts as soon as the first half lands.
            xpt_pool = pa.enter_context(tc.tile_pool(name="xpt", bufs=4))
            xpt_tiles = {}
            xpTr = xpT.rearrange("(k p) n -> p k n", p=128)

            def load_xpt(blk, h):
                t = xpt_pool.tile([128, 8, BN // 2], F16, name="xpt", tag="xpt")
                lo = blk * BN + h * (BN // 2)
                nc.sync.dma_start(t, xpTr[:, :, lo:lo + BN // 2])
                xpt_tiles[(blk, h)] = t

            load_xpt(0, 0)
            nc.sync.dma_start(w1_sb[1], w1r[:, :, C // 2:])
            load_xpt(0, 1)
            # remaining big DMAs, in priority order behind xpt block 0
            wkv_sb = wpool.tile([128, 8, 2 * C], F16, name="wkv_sb")
            nc.sync.dma_start(wkv_sb, wkv.rearrange("(k p) m -> p k m", p=128))
            w2_sb = wpool.tile([128, 8, C], F16, name="w2_sb")
            nc.sync.dma_start(w2_sb, w2.rearrange("(k p) m -> p k m", p=128))
            load_xqt(0)
            load_xqt(1)

            ht_pool = pa.enter_context(tc.tile_pool(name="ht", bufs=1))
            g_pool = pa.enter_context(tc.tile_pool(name="g", bufs=3))
            k_pool = pa.enter_context(tc.tile_pool(name="k", bufs=3))
            vg_pool = pa.enter_context(tc.tile_pool(name="vg", bufs=3))
            g1ps_pool = pa.enter_context(
                tc.tile_pool(name="g1ps", bufs=2, space="PSUM")
            )
            g2ps_pool = pa.enter_context(
                tc.tile_pool(name="g2ps", bufs=2, space="PSUM")
            )
            kvps_pool = pa.enter_context(
                tc.tile_pool(name="kvps", bufs=2, space="PSUM")
            )
            ctps_pool = pa.enter_context(
                tc.tile_pool(name="ctps", bufs=1, space="PSUM")
            )

            # ctx matmuls are emitted one chunk late so their vector-produced
            # inputs (k, vg) are ready by the time PE reaches them.
            pending = []

            def emit_ctx():
                if not pending:
                    return
                k_sb, vg, nch = pending.pop(0)
                ctp = ctps_pool.tile([128, 1024], F32, name="ctp", tag="ctp")
                for j in range(8):
                    nc.tensor.matmul(
                        ctp[:, j * 128:(j + 1) * 128],
                        vg[:, j * 128:(j + 1) * 128],
                        k_sb[:, j * 128:(j + 1) * 128],
                        start=True,
                        stop=True,
                        skip_group_check=True,
                    )
                nc.vector.tensor_add(ctx_acc, ctx_acc, ctp)

            for blk in range(NBLK):
                if blk + 1 < NBLK:
                    load_xpt(blk + 1, 0)
                    load_xpt(blk + 1, 1)
                if blk + 2 < NBLK:
                    load_xqt(blk + 2)
                xpth = [xpt_tiles.pop((blk, h)) for h in range(2)]
                # gate1: hT[m, n] = relu((xp @ w1 + b1).T), w1 stationary
                ht = ht_pool.tile([128, 8, BN], F16, name="ht", tag="ht")
                for m in range(8):
                    for half in range(2):
                        ps = g1ps_pool.tile([128, 512], F32, name="g1ps", tag="g1ps")
                        for kk in range(8):
                            nc.tensor.matmul(
                                ps,
                                w1_sb[m // 4][:, kk, (m % 4) * 128:(m % 4 + 1) * 128],
                                xpth[half][:, kk, :],
                                start=(kk == 0),
                                stop=(kk == 7),
                            )
                        nc.scalar.activation(
                            ht[:, m, half * 512:(half + 1) * 512],
                            ps,
                            AF.Relu,
                            bias=b1_sb[:, m:m + 1],
                        )
                for ch in range(BCH):
                    nch = blk * BCH + ch
                    xpc_t = xpth[ch // 4]
                    xc0 = (ch % 4) * 128
                    # gate2: g[n, q] = sigmoid(h @ w2 + b2), hT stationary
                    g = g_pool.tile([128, C], F16, name="g", tag="g")
                    for half in range(2):
                        ps = g2ps_pool.tile([128, 512], F32, name="g2ps", tag="g2ps")
                        for kk in range(8):
                            nc.tensor.matmul(
                                ps,
                                ht[:, kk, ch * 128:(ch + 1) * 128],
                                w2_sb[:, kk, half * 512:(half + 1) * 512],
                                start=(kk == 0),
                                stop=(kk == 7 and not with_bias),
                            )
                        if with_bias:
                            nc.tensor.matmul(
                                ps,
                                ones_r,
                                b2_r[:, half * 512:(half + 1) * 512],
                                start=False,
                                stop=True,
                            )
                        nc.scalar.activation(
                            g[:, half * 512:(half + 1) * 512], ps, AF.Sigmoid
                        )
                    # kv projection: kv[n, m], xpT stationary
                    k_sb = k_pool.tile([128, C], F16, name="k_sb", tag="k_sb")
                    vg = vg_pool.tile([128, C], F16, name="vg", tag="vg")
                    for q in range(4):
                        ps = kvps_pool.tile([128, 512], F32, name="kvps", tag="kvps")
                        for kk in range(8):
                            nc.tensor.matmul(
                                ps,
                                xpc_t[:, kk, xc0:xc0 + 128],
                                wkv_sb[:, kk, q * 512:(q + 1) * 512],
                                start=(kk == 0),
                                stop=(kk == 7),
                            )
                        if q < 2:
                            nc.vector.tensor_copy(k_sb[:, q * 512:(q + 1) * 512], ps)
                        else:
                            qq = q - 2
                            nc.vector.tensor_mul(
                                vg[:, qq * 512:(qq + 1) * 512],
                                ps,
                                g[:, qq * 512:(qq + 1) * 512],
                            )
                    emit_ctx()
                    pending.append((k_sb, vg, nch))
            emit_ctx()

        # =========================================================
        # Softmax over d (free dim of ctx^T) + build block-diag S pairs
        # =========================================================
        with ExitStack() as sm:
            smp = sm.enter_context(tc.tile_pool(name="smpool", bufs=1))
            smps = sm.enter_context(tc.tile_pool(name="smps", bufs=2, space="PSUM"))
            # softmax chain pipelined in two 512-col halves across DVE/ScalarE
            maxs = smp.tile([128, 16], F32, name="maxs")
            cmx = smp.tile([128, 1024], F32, name="cmx")
            et = smp.tile([128, 1024], F32, name="et")
            sums = smp.tile([128, 16], F32, name="sums")
            recs = smp.tile([128, 16], F32, name="recs")
            stb = smp.tile([128, 1024], F16, name="stb")
            for h in range(2):
                sl = slice(h * 512, (h + 1) * 512)
                gsl = slice(h * 8, (h + 1) * 8)
                nc.vector.tensor_reduce(
                    maxs[:, gsl],
                    ctx_acc[:, sl].rearrange("p (g d) -> p g d", g=8),
                    axis=mybir.AxisListType.X,
                    op=mybir.AluOpType.max,
                )
                nc.vector.tensor_sub(
                    cmx[:, sl].rearrange("p (g d) -> p g d", g=8),
                    ctx_acc[:, sl].rearrange("p (g d) -> p g d", g=8),
                    maxs[:, gsl].unsqueeze(-1).broadcast_to([128, 8, 64]),
                )
                nc.scalar.activation(et[:, sl], cmx[:, sl], AF.Exp,
                                     scale=float(SCALE))
                nc.vector.tensor_reduce(
                    sums[:, gsl],
                    et[:, sl].rearrange("p (g d) -> p g d", g=8),
                    axis=mybir.AxisListType.X,
                    op=mybir.AluOpType.add,
                )
                nc.vector.reciprocal(recs[:, gsl], sums[:, gsl])
                nc.vector.tensor_mul(
                    stb[:, sl].rearrange("p (g d) -> p g d", g=8),
                    et[:, sl].rearrange("p (g d) -> p g d", g=8),
                    recs[:, gsl].unsqueeze(-1).broadcast_to([128, 8, 64]),
                )
                # stb[:, j*128:(j+1)*128] = softmaxed ctxT pair [e(2), d(2)];
                # transpose -> [d(2), e(2)], mask off off-diagonal garbage.
                for j in range(4 * h, 4 * h + 4):
                    tp = smps.tile([128, 128], F16, name="smtp", tag="smtp")
                    nc.tensor.transpose(
                        tp, stb[:, j * 128:(j + 1) * 128], identh_sb
                    )
                    nc.vector.tensor_mul(spairs[j], tp, maskh_sb)

        # =========================================================
        # Phase B (transposed out): oT[j*128:(j+1)*128, nblk] =
        #   spair_j^T @ xqT[j-pair rows, nblk].  spair_j stays stationary
        #   across all n (8 LDWEIGHTS total), xqT streams 1024 cols/MM.
        # =========================================================
        with ExitStack() as pb:
            oo_pool = pb.enter_context(tc.tile_pool(name="oo", bufs=4))
            bops_pool = pb.enter_context(
                tc.tile_pool(name="bops", bufs=4, space="PSUM")
            )
            for j in range(8):
                for blk in range(NBLK):
                    xqt = xqt_tiles[blk]
                    oo = oo_pool.tile([128, BN], F16, name="oo", tag="oo")
                    for h in range(2):
                        ops = bops_pool.tile([128, 512], F32, name="ops",
                                             tag="ops")
                        nc.tensor.matmul(
                            ops,
                            spairs[j],
                            xqt[:, j, h * 512:(h + 1) * 512],
                            start=True,
                            stop=True,
                        )
                        if (blk * 2 + h) % 2 == 0:
                            nc.vector.tensor_copy(
                                oo[:, h * 512:(h + 1) * 512], ops
                            )
                        else:
                            nc.scalar.copy(oo[:, h * 512:(h + 1) * 512], ops)
                    nc.sync.dma_start(
                        o[j * 128:(j + 1) * 128, blk * BN:(blk + 1) * BN], oo
                    )

    nc.compile()
    return nc


def _get_program(with_bias=False):
    key = ("nc", bool(with_bias))
    if key not in _CACHE:
        _CACHE[key] = _build_program(with_bias)
    return _CACHE[key]


def make_in_maps(x1, x2, Wkv1, Wkv2, g1_w1, g1_b1, g1_w2, g1_b2,
                 g2_w1, g2_b1, g2_w2, g2_b2):
    """Core (s, b): cores 0-3 = (s=0, b), cores 4-7 = (s=1, b)."""
    f16 = np.float16
    ident = np.eye(128, dtype=f16)
    mask = np.zeros((128, 128), dtype=f16)
    mask[:64, :64] = np.float16(1.0)
    mask[64:, 64:] = np.float16(1.0)
    # transposed fp16 copies of each batch of each stream (shared across cores)
    x1T = [np.asarray(x1[b], np.float32).T.astype(f16) for b in range(x1.shape[0])]
    x2T = [np.asarray(x2[b], np.float32).T.astype(f16) for b in range(x2.shape[0])]
    wkv1h = np.asarray(Wkv1, np.float32).astype(f16)
    wkv2h = np.asarray(Wkv2, np.float32).astype(f16)
    w11h = np.asarray(g1_w1, np.float32).astype(f16)
    w12h = np.asarray(g1_w2, np.float32).astype(f16)
    w21h = np.asarray(g2_w1, np.float32).astype(f16)
    w22h = np.asarray(g2_w2, np.float32).astype(f16)
    b11 = np.asarray(g1_b1, np.float32)
    b21 = np.asarray(g2_b1, np.float32)
    b12h = np.asarray(g1_b2, np.float32).astype(f16)
    b22h = np.asarray(g2_b2, np.float32).astype(f16)
    in_maps = []
    for core in range(8):
        s, b = core // 4, core % 4
        if s == 0:
            m = dict(xpT=x1T[b], xqT=x2T[b], wkv=wkv1h,
                     w1=w11h, b1=b11, w2=w12h, b2=b12h)
        else:
            m = dict(xpT=x2T[b], xqT=x1T[b], wkv=wkv2h,
                     w1=w21h, b1=b21, w2=w22h, b2=b22h)
        m["identh"] = ident
        m["maskh"] = mask
        in_maps.append(m)
    return in_maps


def kernel(x1, x2, Wkv1, Wkv2, g1_w1, g1_b1, g1_w2, g1_b2,
           g2_w1, g2_b1, g2_w2, g2_b2, _runner=None):
    """Full-input entry point.  Returns (o1, o2), each [4, 4096, 1024] f32."""
    from concourse.bass_utils import run_bass_kernel_spmd

    args = [np.asarray(a, dtype=np.float32) for a in
            (x1, x2, Wkv1, Wkv2, g1_w1, g1_b1, g1_w2, g1_b2,
             g2_w1, g2_b1, g2_w2, g2_b2)]
    with_bias = bool(np.any(args[7]) or np.any(args[11]))  # g1_b2, g2_b2
    nc = _get_program(with_bias)
    in_maps = make_in_maps(*args)
    if _runner is None:
        res = run_bass_kernel_spmd(nc, in_maps, core_ids=list(range(8)))
        results = res.results
    else:
        results = _runner(nc, in_maps)

    B = x1.shape[0]
    o1 = np.empty((B, N, C), dtype=np.float32)
    o2 = np.empty((B, N, C), dtype=np.float32)
    for core in range(8):
        s, b = core // 4, core % 4
        out = np.asarray(results[core]["o"]).T.astype(np.float32)  # [C,N] -> [N,C]
        if s == 0:
            o2[b] = out   # core projected x1 -> ctx1 -> o2 = q2 @ ctx1
        else:
            o1[b] = out
    return (o1, o2)
